# revision 1
# baseline (speedup 1.0000x reference)
"""Trainium2 Bass kernel for nn_DecoupledTextDecoder.

Reference computation (per batch sample b, nB=256, nC=512, nH*nW=512, nT=40,
nCls=97):
  A_n   = A / sum_hw(A)                       (attention normalize)
  C     = einsum('chw,thw->tc', feature_b, A_n_b)       [40, 512]
  hidden= C @ W.T + b                                   [40, 512]
  cfP   = hidden @ protos.T                             [40, 97]
  cfCos = cfP / (||hidden||_row + EPS)
  outCls= concat([cfP * ALPHA, UNK], -1); outCos = concat([cfCos, UNK], -1)
  ragged-pack the first textLength[b] rows of each sample into one buffer.

Strategy: data-parallel over nB across 8 NeuronCores (32 samples/core).
The hw-contraction needs hw on SBUF partitions for the PE, so feature and A
are uploaded pre-transposed ([b, hw, c] / [b, hw, t]) — a host-side layout
choice — removing all on-chip transposes.  The attention normalization is
algebraically folded into a per-column scale s[t]=1/rowsum(A) applied to C
(host computes s exactly in fp32).  Per-sample matmul chain on the PE:
  M1: C^T[c,t]      = FT-chunks(lhsT) x AT-chunks(rhs), accumulated over hw
  M2: hidden^T[c',t]= WT-chunks(lhsT) x C^T(rhs),       accumulated over c
  M3: cfP[t,cls]    = hidden^T-slices(lhsT) x protos^T(rhs), accum over c'
  hnorm^2 via DVE squares + ones-matmul partition reduce -> [t,1] layout,
  so the cfCos division and ALPHA scale are per-partition tensor_scalars.
The ragged pack is pure data movement with runtime row offsets; it is done
on the host with a vectorized scatter (the per-sample rows shard cleanly).

Matmul operands use fp16 (inputs rounded on host / on copy); accumulation
is fp32 in PSUM and everything after M3 stays fp32.  Measured end-to-end
resid-var vs the fp32 reference is ~1e-7.
"""

import numpy as np

import concourse.bass as bass
import concourse.bacc as bacc
import concourse.tile as tile
import concourse.mybir as mybir
from concourse.bass_utils import run_bass_kernel_spmd

F32 = mybir.dt.float32
EPS = 0.0009

N_CORES = 8
NB = 256
NB_C = NB // N_CORES       # samples per core
NC = 512                   # channels
HW = 512                   # nH*nW
NT = 40                    # text steps
NCLS = 97
D = NCLS + 1

GROUP = 2                  # samples per feature DMA (1 MiB transfers)
BLOCKS = [12, 12, 8]       # samples per block (sum = NB_C)
assert sum(BLOCKS) == NB_C and all(b % GROUP == 0 for b in BLOCKS)


def _mgroups(ns):
    """Partition-dim groups of t-columns for M3 (<=128 each, 40-aligned)."""
    w = ns * NT
    out, o = [], 0
    while o < w:
        m = min(120, w - o)
        out.append((o, m))
        o += m
    return out


def build_kernel(dt2=mybir.dt.float16, reps=1, group=GROUP, dual_ring=False,
                 ft_bufs=3, timing_mode=False, hw_loop=0, rings=None,
                 out_rings=None, skip_load=False, skip_compute=False,
                 dup_dma=False, dup_m1=False, at_g=False, flat_ft=False,
                 at_hoist=False, split2=False, dt_ft=None):
    """Build + compile the per-core Bass program. Returns nc.

    timing_mode=True replaces the bulk inputs with Internal DRAM scratch so
    repeated-execution benchmarks don't pay host->device re-transfers; the
    on-device HBM traffic is identical.
    """
    nc = bacc.Bacc("TRN2", target_bir_lowering=False, debug=False,
                   enable_asserts=True, num_devices=N_CORES)

    if dt_ft is None:
        dt_ft = dt2
    kind_b = "Internal" if timing_mode else "ExternalInput"
    ft = nc.dram_tensor("ft", [NB_C * HW, NC], dt_ft, kind=kind_b).ap()
    at = nc.dram_tensor("at", [NB_C * HW, NT], dt_ft, kind=kind_b).ap()
    wt = nc.dram_tensor("wt", [NC, NC], dt2, kind=kind_b).ap()
    pt = nc.dram_tensor("pt", [NC, NCLS], dt2, kind=kind_b).ap()
    bcol = nc.dram_tensor("bcol", [128, NC // 128], F32, kind=kind_b).ap()
    srow = nc.dram_tensor("srow", [1, NB_C * NT], F32, kind=kind_b).ap()
    au = nc.dram_tensor("au", [1, 2], F32, kind="ExternalInput").ap()
    ocls = nc.dram_tensor("ocls", [NB_C * NT, D], F32, kind="ExternalOutput").ap()
    ocos = nc.dram_tensor("ocos", [NB_C * NT, D], F32, kind="ExternalOutput").ap()

    with tile.TileContext(nc) as tc:
        with (
            tc.tile_pool(name="consts", bufs=1) as consts,
            tc.tile_pool(name="ftp", bufs=ft_bufs) as ftp,
            tc.tile_pool(name="atp", bufs=3) as atp,
            tc.tile_pool(name="work", bufs=8) as work,
            tc.tile_pool(name="sqp", bufs=5) as sqp,
            tc.tile_pool(name="outp", bufs=12) as outp,
            tc.tile_pool(name="ps_ct", bufs=4, space="PSUM") as ps_ct,
            tc.tile_pool(name="ps_h", bufs=2, space="PSUM") as ps_h,
            tc.tile_pool(name="ps_p", bufs=1, space="PSUM") as ps_p,
            tc.tile_pool(name="ps_sq", bufs=1, space="PSUM") as ps_sq,
        ):
            if rings is None:
                ring_eng = [nc.scalar, nc.sync] if dual_ring else [nc.sync]
            else:
                emap = {"s": nc.sync, "a": nc.scalar, "g": nc.gpsimd}
                ring_eng = [emap[ch] for ch in rings]
            if out_rings is None:
                oring_eng = [nc.sync]
            else:
                emap = {"s": nc.sync, "a": nc.scalar, "g": nc.gpsimd}
                oring_eng = [emap[ch] for ch in out_rings]

            def emit():
                _emit_once(nc, tc, consts, ftp, atp, work, sqp, outp,
                           ps_ct, ps_h, ps_p, ps_sq,
                           ft, at, wt, pt, bcol, srow, au, ocls, ocos, dt2,
                           group, ring_eng, oring_eng, skip_load, skip_compute,
                           dup_dma, dup_m1, at_g, flat_ft, at_hoist, split2,
                           dt_ft)

            if hw_loop:
                with tc.For_i(0, hw_loop, 1):
                    emit()
            else:
                for _ in range(reps):
                    emit()
    nc.compile()
    return nc


def _emit_once(nc, tc, consts, ftp, atp, work, sqp, outp,
               ps_ct, ps_h, ps_p, ps_sq,
               ft, at, wt, pt, bcol, srow, au, ocls, ocos, dt2,
               group, ring_eng, oring_eng, skip_load=False, skip_compute=False,
               dup_dma=False, dup_m1=False, at_g=False, flat_ft=False,
               at_hoist=False, split2=False, dt_ft=None):
    if dt_ft is None:
        dt_ft = dt2
    mult = mybir.AluOpType.mult
    add = mybir.AluOpType.add

    # ---- constants -------------------------------------------------------
    ones_row = consts.tile([1, 128], F32, tag="ones_row")
    nc.vector.memset(ones_row[:], 1.0)
    ones_col = consts.tile([128, 1], F32, tag="ones_col")
    nc.vector.memset(ones_col[:], 1.0)

    wt_sb = []
    for k in range(4):
        t = consts.tile([128, NC], dt2, tag=f"wt{k}")
        nc.sync.dma_start(out=t[:], in_=wt[k * 128:(k + 1) * 128, :])
        wt_sb.append(t)
    pt_sb = []
    for k in range(4):
        t = consts.tile([128, NCLS], dt2, tag=f"pt{k}")
        nc.sync.dma_start(out=t[:], in_=pt[k * 128:(k + 1) * 128, :])
        pt_sb.append(t)
    b_sb = consts.tile([128, 4], F32, tag="b")
    nc.sync.dma_start(out=b_sb[:], in_=bcol[:])
    s_sb = consts.tile([1, NB_C * NT], F32, tag="s")
    nc.sync.dma_start(out=s_sb[:], in_=srow[:])
    au_sb = consts.tile([1, 2], F32, tag="au")
    nc.sync.dma_start(out=au_sb[:], in_=au[:])

    # Broadcast s over partitions via k=1 matmul: S_all[p, col] = s[col].
    s_all = consts.tile([128, NB_C * NT], F32, tag="s_all")
    o = 0
    while o < NB_C * NT:
        w = min(512, NB_C * NT - o)
        ps = ps_h.tile([128, 512], F32, tag="h")
        nc.tensor.matmul(ps[:, :w], ones_row[:], s_sb[:, o:o + w],
                         start=True, stop=True)
        nc.vector.tensor_copy(s_all[:, o:o + w], ps[:, :w])
        o += w
    # alpha / unk broadcast columns
    au_ps = ps_h.tile([128, 512], F32, tag="h")
    nc.tensor.matmul(au_ps[:, :2], ones_row[:], au_sb[:], start=True, stop=True)
    au_col = consts.tile([128, 2], F32, tag="au_col")
    nc.vector.tensor_copy(au_col[:], au_ps[:, :2])
    alpha_col = au_col[:, 0:1]
    unk_col = au_col[:, 1:2]

    at_all = None
    if at_hoist:
        at_all = atp.tile([128, NB_C * 4, NT], dt_ft, tag="at_all")
        half = NB_C * HW // 2
        ring_eng[0].dma_start(
            out=at_all[:, :NB_C * 2, :],
            in_=at[0:half, :].rearrange("(g p) t -> p g t", p=128))
        ring_eng[-1].dma_start(
            out=at_all[:, NB_C * 2:, :],
            in_=at[half:NB_C * HW, :].rearrange("(g p) t -> p g t", p=128))

    # ---- main loop over sample blocks -----------------------------------
    s0 = 0
    for ns in BLOCKS:
        w = ns * NT
        col0 = s0 * NT

        # feature/attention loads, `group` samples per DMA
        ftg, atg = [], []
        for j in range(ns // group):
            r0 = (s0 + j * group) * HW
            eng = ring_eng[j % len(ring_eng)]
            ftile = ftp.tile([128, group * 4, NC], dt_ft, tag="ft")
            atile = atp.tile([128, group * 4, NT], dt_ft, tag="at")
            if not skip_load:
                if split2:
                    hwrows = group * HW // 2
                    ring_eng[0].dma_start(
                        out=ftile[:, :group * 2, :],
                        in_=ft[r0:r0 + hwrows, :].rearrange("(g p) c -> p g c", p=128))
                    ring_eng[-1].dma_start(
                        out=ftile[:, group * 2:, :],
                        in_=ft[r0 + hwrows:r0 + group * HW, :].rearrange("(g p) c -> p g c", p=128))
                elif flat_ft:
                    for q in range(group * 4):
                        ring_eng[(j * group * 4 + q) % len(ring_eng)].dma_start(
                            out=ftile[:, q, :],
                            in_=ft[r0 + q * 128:r0 + (q + 1) * 128, :])
                else:
                    eng.dma_start(
                        out=ftile[:],
                        in_=ft[r0:r0 + group * HW, :].rearrange("(g p) c -> p g c", p=128))
                if not at_hoist:
                    (nc.gpsimd if at_g else eng).dma_start(
                        out=atile[:],
                        in_=at[r0:r0 + group * HW, :].rearrange("(g p) t -> p g t", p=128))
                if dup_dma:
                    dtile = ftp.tile([128, group * 4, NC], dt_ft, tag="ftdup", name="dtile")
                    eng.dma_start(
                        out=dtile[:],
                        in_=ft[r0:r0 + group * HW, :].rearrange("(g p) c -> p g c", p=128))
            ftg.append(ftile)
            atg.append(atile)
        if skip_compute:
            s0 += ns
            continue

        # M1: C_raw^T accumulated into 4 psum banks, one 40-col slice/sample
        ct_ps = [ps_ct.tile([128, 480], F32, tag="ct", name=f"ct_ps{jj}") for jj in range(4)]
        for sl in range(ns):
            ftile = ftg[sl // group]
            h = sl % group
            if at_hoist:
                atile, abase = at_all, (s0 + sl) * 4
            else:
                atile, abase = atg[sl // group], (sl % group) * 4
            for rep2 in range(2 if dup_m1 else 1):
                for jj in range(4):
                    for kk in range(4):
                        nc.tensor.matmul(
                            ct_ps[jj][:, sl * NT:(sl + 1) * NT],
                            ftile[:, h * 4 + kk, jj * 128:(jj + 1) * 128],
                            atile[:, abase + kk, :],
                            start=(kk == 0), stop=(kk == 3))

        # scale by s (normalization fold) + cast to dt2
        ct_sb = []
        for jj in range(4):
            t = work.tile([128, 480], dt2, tag="ctsb")
            nc.vector.tensor_tensor(t[:, :w], ct_ps[jj][:, :w],
                                    s_all[:, col0:col0 + w], mult)
            ct_sb.append(t)

        # M2: hidden^T (no bias yet), 4 psum banks
        h_sb = []
        sq = []
        for jj in range(4):
            hp = ps_h.tile([128, 480], F32, tag="h")
            for kk in range(4):
                nc.tensor.matmul(hp[:, :w], wt_sb[kk][:, jj * 128:(jj + 1) * 128],
                                 ct_sb[kk][:, :w], start=(kk == 0), stop=(kk == 3))
            # bias add on ACT during psum->sbuf copy (cast to dt2)
            hs = work.tile([128, 480], dt2, tag="hsb")
            nc.scalar.activation(hs[:, :w], hp[:, :w],
                                 mybir.ActivationFunctionType.Identity,
                                 bias=b_sb[:, jj:jj + 1])
            h_sb.append(hs)
            # squared hidden for the row norms
            st = sqp.tile([128, 480], F32, tag="sq")
            nc.vector.tensor_tensor(st[:, :w], hs[:, :w], hs[:, :w], mult)
            sq.append(st)
        nc.vector.tensor_tensor(sq[0][:, :w], sq[0][:, :w], sq[1][:, :w], add)
        nc.vector.tensor_tensor(sq[2][:, :w], sq[2][:, :w], sq[3][:, :w], add)
        nc.vector.tensor_tensor(sq[0][:, :w], sq[0][:, :w], sq[2][:, :w], add)

        mg = _mgroups(ns)
        # partition-reduce -> hnorm^2 in [t, 1] layout
        sq_ps = ps_sq.tile([128, len(mg)], F32, tag="sqc")
        for g, (o, m) in enumerate(mg):
            nc.tensor.matmul(sq_ps[:m, g:g + 1], sq[0][:, o:o + m], ones_col[:],
                             start=True, stop=True)
        # r = 1 / (sqrt(hnorm^2) + EPS)
        rcols = work.tile([128, len(mg)], F32, tag="rc")
        for g, (o, m) in enumerate(mg):
            nc.scalar.sqrt(rcols[:m, g:g + 1], sq_ps[:m, g:g + 1])
            nc.vector.tensor_scalar_add(rcols[:m, g:g + 1], rcols[:m, g:g + 1], EPS)
            nc.vector.reciprocal(rcols[:m, g:g + 1], rcols[:m, g:g + 1])

        # M3 + outputs
        p_ps = ps_p.tile([128, len(mg) * NCLS], F32, tag="p")
        for g, (o, m) in enumerate(mg):
            for kk in range(4):
                nc.tensor.matmul(p_ps[:m, g * NCLS:(g + 1) * NCLS],
                                 h_sb[kk][:, o:o + m], pt_sb[kk][:],
                                 start=(kk == 0), stop=(kk == 3))
            oc = outp.tile([128, D], F32, tag="ocls")
            nc.vector.tensor_scalar(oc[:m, 0:NCLS], p_ps[:m, g * NCLS:(g + 1) * NCLS],
                                    alpha_col[:m, :], None, mult)
            nc.vector.tensor_copy(oc[:m, NCLS:D], unk_col[:m, :])
            oring_eng[g % len(oring_eng)].dma_start(
                out=ocls[col0 + o:col0 + o + m, :], in_=oc[:m, :])

            os_ = outp.tile([128, D], F32, tag="ocos")
            nc.vector.tensor_scalar(os_[:m, 0:NCLS], p_ps[:m, g * NCLS:(g + 1) * NCLS],
                                    rcols[:m, g:g + 1], None, mult)
            nc.vector.tensor_copy(os_[:m, NCLS:D], unk_col[:m, :])
            oring_eng[(g + 1) % len(oring_eng)].dma_start(
                out=ocos[col0 + o:col0 + o + m, :], in_=os_[:m, :])
        s0 += ns


def host_prep(feature, A, protos, W, b, ALPHA, UNK_SCR, np_dt=np.float16):
    """Build the 8 per-core input maps (host-side layout prep)."""
    f3 = np.ascontiguousarray(feature.reshape(NB, NC, HW).transpose(0, 2, 1)).astype(np_dt)
    a3r = A.reshape(NB, NT, HW)
    a3 = np.ascontiguousarray(a3r.transpose(0, 2, 1)).astype(np_dt)
    s = (1.0 / a3r.sum(axis=2, dtype=np.float64)).astype(np.float32)  # [NB, NT]
    wt = np.ascontiguousarray(W.T).astype(np_dt)
    pt = np.ascontiguousarray(protos.T).astype(np_dt)
    bcol = np.ascontiguousarray(b.reshape(4, 128).T).astype(np.float32)
    au = np.array([[float(ALPHA[0, 0]), float(UNK_SCR[0, 0])]], np.float32)
    in_maps = []
    for c in range(N_CORES):
        sl = slice(c * NB_C, (c + 1) * NB_C)
        in_maps.append(dict(
            ft=f3[sl].reshape(NB_C * HW, NC),
            at=a3[sl].reshape(NB_C * HW, NT),
            wt=wt, pt=pt, bcol=bcol,
            srow=s[sl].reshape(1, NB_C * NT),
            au=au,
        ))
    return in_maps


def host_pack(dense_cls, dense_cos, textLength):
    """Ragged per-sample packing (matches reference.pack)."""
    usedLen = np.minimum(textLength.astype(np.int64), NT)
    offsets = np.cumsum(usedLen) - usedLen
    b_idx, t_idx = np.nonzero(t_mask := (np.arange(NT)[None, :] < usedLen[:, None]))
    out_cls = np.zeros((NB * NT, D), np.float32)
    out_cos = np.zeros((NB * NT, D), np.float32)
    dest = offsets[b_idx] + t_idx
    src = b_idx * NT + t_idx
    out_cls[dest] = dense_cls[src]
    out_cos[dest] = dense_cos[src]
    return out_cls, out_cos


_NC_CACHE = {}


def _get_nc(dt2=mybir.dt.float16, reps=1, **kw):
    key = (str(dt2), reps, tuple(sorted(kw.items())))
    if key not in _NC_CACHE:
        _NC_CACHE[key] = build_kernel(dt2, reps, **kw)
    return _NC_CACHE[key]


FINAL_CFG = dict(dual_ring=True, out_rings="sa")


def kernel(feature, A, protos, W, b, ALPHA, UNK_SCR, textLength):
    feature = np.asarray(feature, np.float32)
    A = np.asarray(A, np.float32)
    in_maps = host_prep(np.asarray(feature, np.float32), np.asarray(A, np.float32),
                        np.asarray(protos, np.float32), np.asarray(W, np.float32),
                        np.asarray(b, np.float32), np.asarray(ALPHA, np.float32),
                        np.asarray(UNK_SCR, np.float32))
    nc = _get_nc(**FINAL_CFG)
    res = None
    for attempt in range(3):
        try:
            res = run_bass_kernel_spmd(nc, in_maps, core_ids=list(range(N_CORES)))
            break
        except Exception:  # noqa: BLE001 - transient device/tunnel hiccups
            if attempt == 2:
                raise
            import time as _time
            _time.sleep(30)
    dense_cls = np.concatenate([res.results[c]["ocls"] for c in range(N_CORES)], axis=0)
    dense_cos = np.concatenate([res.results[c]["ocos"] for c in range(N_CORES)], axis=0)
    return host_pack(dense_cls, dense_cos, np.asarray(textLength))



# revision 2
# speedup vs baseline: 2.4615x; 2.4615x over previous
"""Trainium2 Bass kernel for nn_DecoupledTextDecoder.

Reference computation (per batch sample b, nB=256, nC=512, nH*nW=512, nT=40,
nCls=97):
  A_n   = A / sum_hw(A)                       (attention normalize)
  C     = einsum('chw,thw->tc', feature_b, A_n_b)       [40, 512]
  hidden= C @ W.T + b                                   [40, 512]
  cfP   = hidden @ protos.T                             [40, 97]
  cfCos = cfP / (||hidden||_row + EPS)
  outCls= concat([cfP * ALPHA, UNK], -1); outCos = concat([cfCos, UNK], -1)
  ragged-pack the first textLength[b] rows of each sample into one buffer.

Strategy: data-parallel over nB across 8 NeuronCores (32 samples/core).
Host-side algebraic folds shrink the device program to three matmul stages
and one activation pass:
  * The attention normalization is folded into A on host:
    at = fp8(256 * A / sum_hw(A)); the 1/256 is folded into W/protos.
  * M1: Craw^T[c,t] = ft-chunks(lhsT) x at-chunks(rhs), accum over hw.
    Both operands fp8 (e4m3) - halves the dominant HBM traffic.
  * M2: h0[t,c'] = ct-slices(lhsT) x (W.T/256)-chunks(rhs), accum over c.
    One Square-activation pass per row-group with accum_out gives
    ||h0||^2 per t-row directly (hidden itself is never stored).
  * M3: ct-slices(lhsT) x P2-chunks(rhs) where P2 = [(W.T@protos.T)/256,
    (W.T@b)/256]: col 0..96 = cfP-without-bias, col 97 = h0.b per row.
  * Host finishes: cfP += protos@b; hnorm^2 = h2 + 2*h0.b + ||b||^2;
    ALPHA/cos scaling, UNK column, and the ragged pack (pure data
    movement with runtime offsets) are all host-side.
All DRAM operands are host-pre-arranged so every DMA is contiguous per
partition (8KB+ descriptors); outputs consolidate into one [128, 11*99]
fp32 tensor written with 3 block-level DMAs.
"""

import numpy as np
import ml_dtypes

import concourse.bass as bass
import concourse.bacc as bacc
import concourse.tile as tile
import concourse.mybir as mybir
from concourse.bass_utils import run_bass_kernel_spmd

F32 = mybir.dt.float32
F16 = mybir.dt.float16
F8 = mybir.dt.float8e4
EPS = 0.0009

N_CORES = 8
NB = 256
NB_C = NB // N_CORES       # samples per core
NC = 512                   # channels
HW = 512                   # nH*nW
NT = 40                    # text steps
NCLS = 97
D = NCLS + 1

ASCALE = 256.0             # host scale on normalized A; 1/ASCALE folded into W
BLOCKS = [12, 12, 8]       # samples per block (sum = NB_C)
GROUP = 4                  # samples per feature DMA


def _mgroups(ns):
    """Row-groups of t-columns (<=120 each) for the per-group stages."""
    w = ns * NT
    out, o = [], 0
    while o < w:
        m = min(120, w - o)
        out.append((o, m))
        o += m
    return out


GTOT = sum(len(_mgroups(ns)) for ns in BLOCKS)   # 11 output groups
OCOLS = GTOT * (D + 1)                           # 99 cols per group


def build_kernel(reps=1, group=GROUP, timing_mode=False, hw_loop=0,
                 dt_in="f8", rings="sa", out_rings="sa", ft_bufs=3):
    """Build + compile the per-core Bass program. Returns nc.

    timing_mode=True replaces the bulk inputs with Internal DRAM scratch so
    repeated-execution benchmarks don't pay host->device re-transfers; the
    on-device HBM traffic is identical.
    """
    nc = bacc.Bacc("TRN2", target_bir_lowering=False, debug=False,
                   enable_asserts=True, num_devices=N_CORES)
    dt_ft = {"f8": F8, "f16": F16}[dt_in]

    kind_b = "Internal" if timing_mode else "ExternalInput"
    ft = nc.dram_tensor("ft", [128, NB_C * 4 * NC], dt_ft, kind=kind_b).ap()
    at = nc.dram_tensor("at", [128, NB_C * 4 * NT], dt_ft, kind=kind_b).ap()
    wt = nc.dram_tensor("wt", [128, 4 * NC], F16, kind=kind_b).ap()
    p2 = nc.dram_tensor("p2", [128, 4 * D], F16, kind=kind_b).ap()
    oraw = nc.dram_tensor("oraw", [128, OCOLS], F32, kind="ExternalOutput").ap()

    with tile.TileContext(nc) as tc:
        with (
            tc.tile_pool(name="consts", bufs=1) as consts,
            tc.tile_pool(name="ftp", bufs=ft_bufs) as ftp,
            tc.tile_pool(name="ctp", bufs=8) as ctp,
            tc.tile_pool(name="sqp", bufs=2) as sqp,
            tc.tile_pool(name="outp", bufs=3) as outp,
            tc.tile_pool(name="ps_ct", bufs=4, space="PSUM") as ps_ct,
            tc.tile_pool(name="ps_h", bufs=2, space="PSUM") as ps_h,
            tc.tile_pool(name="ps_p", bufs=2, space="PSUM") as ps_p,
        ):
            emap = {"s": nc.sync, "a": nc.scalar, "g": nc.gpsimd}
            ring_eng = [emap[ch] for ch in rings]
            oring_eng = [emap[ch] for ch in out_rings]

            def emit():
                _emit_once(nc, tc, consts, ftp, ctp, sqp, outp,
                           ps_ct, ps_h, ps_p,
                           ft, at, wt, p2, oraw, dt_ft,
                           group, ring_eng, oring_eng)

            if hw_loop:
                with tc.For_i(0, hw_loop, 1):
                    emit()
            else:
                for _ in range(reps):
                    emit()
    nc.compile()
    return nc


def _emit_once(nc, tc, consts, ftp, ctp, sqp, outp,
               ps_ct, ps_h, ps_p,
               ft, at, wt, p2, oraw, dt_ft, group, ring_eng, oring_eng):
    # ---- constants / hoisted loads --------------------------------------
    wt_sb = consts.tile([128, 4 * NC], F16, tag="wt")
    ring_eng[0].dma_start(out=wt_sb[:], in_=wt[:])
    p2_sb = consts.tile([128, 4 * D], F16, tag="p2")
    ring_eng[-1].dma_start(out=p2_sb[:], in_=p2[:])

    at_all = consts.tile([128, NB_C * 4, NT], dt_ft, tag="at_all")
    half = NB_C * 2
    ring_eng[0].dma_start(out=at_all[:, :half, :], in_=at[:, :half * NT])
    ring_eng[-1].dma_start(out=at_all[:, half:, :], in_=at[:, half * NT:])

    # ---- main loop over sample blocks -----------------------------------
    s0 = 0
    gi = 0          # global output-group index
    ji = 0          # feature-DMA ring rotation
    for ns in BLOCKS:
        w = ns * NT

        # feature loads, `group` samples per DMA, contiguous per partition
        ftg = []
        for j in range(ns // group):
            off = (s0 + j * group) * 4 * NC
            ftile = ftp.tile([128, group * 4, NC], dt_ft, tag="ft")
            ring_eng[ji % len(ring_eng)].dma_start(
                out=ftile[:], in_=ft[:, off:off + group * 4 * NC])
            ji += 1
            ftg.append(ftile)

        # M1: Craw^T accumulated into 4 psum banks, one 40-col slice/sample
        ct_ps = [ps_ct.tile([128, 512], F32, tag="ct", name=f"ct_ps{jj}")
                 for jj in range(4)]
        for sl in range(ns):
            ftile = ftg[sl // group]
            h = sl % group
            abase = (s0 + sl) * 4
            for jj in range(4):
                for kk in range(4):
                    nc.tensor.matmul(
                        ct_ps[jj][:, sl * NT:(sl + 1) * NT],
                        ftile[:, h * 4 + kk, jj * 128:(jj + 1) * 128],
                        at_all[:, abase + kk, :],
                        start=(kk == 0), stop=(kk == 3))

        # cast to fp16 for the downstream matmuls
        ct_sb = []
        for jj in range(4):
            t = ctp.tile([128, 480], F16, tag="ct_sb")
            nc.vector.tensor_copy(t[:, :w], ct_ps[jj][:, :w])
            ct_sb.append(t)

        odense = outp.tile([128, 4 * (D + 1)], F32, tag="od")
        ng = 0
        for (o, m) in _mgroups(ns):
            col = ng * (D + 1)
            # M2: h0[t, c'] for this row-group; only its row-norms survive
            h2_ps = ps_h.tile([128, 512], F32, tag="h2")
            for kk in range(4):
                nc.tensor.matmul(h2_ps[:m, :], ct_sb[kk][:, o:o + m],
                                 wt_sb[:, kk * NC:(kk + 1) * NC],
                                 start=(kk == 0), stop=(kk == 3))
            sq = sqp.tile([128, 512], F32, tag="sq")
            nc.scalar.activation(sq[:m, :], h2_ps[:m, :],
                                 mybir.ActivationFunctionType.Square,
                                 accum_out=odense[:m, col + D:col + D + 1])

            # M3: cfP-without-bias (97 cols) + h0.b (col 97)
            p_ps = ps_p.tile([128, D], F32, tag="p")
            for kk in range(4):
                nc.tensor.matmul(p_ps[:m, :], ct_sb[kk][:, o:o + m],
                                 p2_sb[:, kk * D:(kk + 1) * D],
                                 start=(kk == 0), stop=(kk == 3))
            nc.vector.tensor_copy(odense[:m, col:col + D], p_ps[:m, :])
            ng += 1

        oring_eng[gi % len(oring_eng)].dma_start(
            out=oraw[:, gi * (D + 1):(gi + ng) * (D + 1)],
            in_=odense[:, :ng * (D + 1)])
        gi += ng
        s0 += ns


# ---- host side -----------------------------------------------------------

def host_prep(feature, A, protos, W, b, np_dt=ml_dtypes.float8_e4m3):
    """Build the 8 per-core input maps (host-side layout + algebra prep)."""
    f3 = feature.reshape(NB, NC, HW).transpose(0, 2, 1)      # [NB, HW, NC]
    a3 = A.reshape(NB, NT, HW)
    s = a3.sum(axis=2, dtype=np.float64)                     # [NB, NT]
    a_n = (ASCALE * a3 / s[:, :, None]).astype(np.float32)
    a_n = a_n.transpose(0, 2, 1)                             # [NB, HW, NT]

    wt2 = (W.T.astype(np.float64) / ASCALE)                  # [c, c']
    wt_dev = wt2.reshape(4, 128, NC).transpose(1, 0, 2).reshape(128, 4 * NC)
    p2t = wt2 @ protos.T.astype(np.float64)                  # [c, 97]
    hbcol = wt2 @ b.astype(np.float64)                       # [c]
    p2full = np.concatenate([p2t, hbcol[:, None]], axis=1)   # [c, 98]
    p2_dev = p2full.reshape(4, 128, D).transpose(1, 0, 2).reshape(128, 4 * D)

    in_maps = []
    for c in range(N_CORES):
        sl = slice(c * NB_C, (c + 1) * NB_C)
        fc = f3[sl].reshape(NB_C, 4, 128, NC).transpose(2, 0, 1, 3)
        ac = a_n[sl].reshape(NB_C, 4, 128, NT).transpose(2, 0, 1, 3)
        in_maps.append(dict(
            ft=np.ascontiguousarray(fc.reshape(128, NB_C * 4 * NC)).astype(np_dt),
            at=np.ascontiguousarray(ac.reshape(128, NB_C * 4 * NT)).astype(np_dt),
            wt=wt_dev.astype(np.float16),
            p2=p2_dev.astype(np.float16),
        ))
    return in_maps


def host_finish(oraws, protos, W, b, ALPHA, UNK_SCR):
    """Decode the per-core raw outputs into dense [NB*NT, D] cls/cos."""
    pb = (protos.astype(np.float64) @ b.astype(np.float64)).astype(np.float32)
    bb = float(b.astype(np.float64) @ b.astype(np.float64))
    alpha = float(np.asarray(ALPHA).reshape(-1)[0])
    unk = float(np.asarray(UNK_SCR).reshape(-1)[0])

    groups = []     # (flat_row_base, m, gi)
    s0, gi = 0, 0
    for ns in BLOCKS:
        for (o, m) in _mgroups(ns):
            groups.append((s0 * NT + o, m, gi))
            gi += 1
        s0 += ns

    dense_cls = np.empty((NB * NT, D), np.float32)
    dense_cos = np.empty((NB * NT, D), np.float32)
    for c, raw in enumerate(oraws):
        r = raw.reshape(128, GTOT, D + 1)
        base_c = c * NB_C * NT
        for row0, m, g in groups:
            cf = r[:m, g, :NCLS] + pb[None, :]
            hb = r[:m, g, NCLS]
            h2 = r[:m, g, NCLS + 1]
            hnorm = np.sqrt(np.maximum(h2 + 2.0 * hb + bb, 0.0))
            rows = slice(base_c + row0, base_c + row0 + m)
            dense_cls[rows, :NCLS] = cf * alpha
            dense_cos[rows, :NCLS] = cf / (hnorm[:, None] + EPS)
    dense_cls[:, NCLS] = unk
    dense_cos[:, NCLS] = unk
    return dense_cls, dense_cos


def host_pack(dense_cls, dense_cos, textLength):
    """Ragged per-sample packing (matches reference.pack)."""
    usedLen = np.minimum(textLength.astype(np.int64), NT)
    offsets = np.cumsum(usedLen) - usedLen
    b_idx, t_idx = np.nonzero(np.arange(NT)[None, :] < usedLen[:, None])
    out_cls = np.zeros((NB * NT, D), np.float32)
    out_cos = np.zeros((NB * NT, D), np.float32)
    dest = offsets[b_idx] + t_idx
    src = b_idx * NT + t_idx
    out_cls[dest] = dense_cls[src]
    out_cos[dest] = dense_cos[src]
    return out_cls, out_cos


_NC_CACHE = {}


def _get_nc(**kw):
    key = tuple(sorted(kw.items()))
    if key not in _NC_CACHE:
        _NC_CACHE[key] = build_kernel(**kw)
    return _NC_CACHE[key]


FINAL_CFG = dict(dt_in="f8", rings="sa", out_rings="sa")


def kernel(feature, A, protos, W, b, ALPHA, UNK_SCR, textLength):
    feature = np.asarray(feature, np.float32)
    A = np.asarray(A, np.float32)
    protos = np.asarray(protos, np.float32)
    W = np.asarray(W, np.float32)
    b = np.asarray(b, np.float32)
    in_maps = host_prep(feature, A, protos, W, b)
    nc = _get_nc(**FINAL_CFG)
    res = None
    for attempt in range(3):
        try:
            res = run_bass_kernel_spmd(nc, in_maps, core_ids=list(range(N_CORES)))
            break
        except Exception:  # noqa: BLE001 - transient device/tunnel hiccups
            if attempt == 2:
                raise
            import time as _time
            _time.sleep(30)
    oraws = [res.results[c]["oraw"] for c in range(N_CORES)]
    dense_cls, dense_cos = host_finish(oraws, protos, W, b, ALPHA, UNK_SCR)
    return host_pack(dense_cls, dense_cos, np.asarray(textLength))


# revision 6
# speedup vs baseline: 2.7662x; 1.1238x over previous
"""Trainium2 Bass kernel for nn_DecoupledTextDecoder.

Reference computation (per batch sample b, nB=256, nC=512, nH*nW=512, nT=40,
nCls=97):
  A_n   = A / sum_hw(A)                       (attention normalize)
  C     = einsum('chw,thw->tc', feature_b, A_n_b)       [40, 512]
  hidden= C @ W.T + b                                   [40, 512]
  cfP   = hidden @ protos.T                             [40, 97]
  cfCos = cfP / (||hidden||_row + EPS)
  outCls= concat([cfP * ALPHA, UNK], -1); outCos = concat([cfCos, UNK], -1)
  ragged-pack the first textLength[b] rows of each sample into one buffer.

Strategy: data-parallel over nB across 8 NeuronCores (32 samples/core).
Host-side algebraic folds shrink the device program to three matmul stages
and one activation pass:
  * The attention normalization is folded into A on host:
    at = fp8(256 * A / sum_hw(A)); the 1/256 is folded into W/protos.
  * M1: Craw^T[c,t] = ft-chunks(lhsT) x at-chunks(rhs), accum over hw.
    Both operands fp8 (e4m3) - halves the dominant HBM traffic.
  * M2: h0[t,c'] = ct-slices(lhsT) x (W.T/256)-chunks(rhs), accum over c.
    One Square-activation pass per row-group with accum_out gives
    ||h0||^2 per t-row directly (hidden itself is never stored).
  * M3: ct-slices(lhsT) x P2-chunks(rhs) where P2 = [(W.T@protos.T)/256,
    (W.T@b)/256]: col 0..96 = cfP-without-bias, col 97 = h0.b per row.
  * Host finishes: cfP += protos@b; hnorm^2 = h2 + 2*h0.b + ||b||^2;
    ALPHA/cos scaling, UNK column, and the ragged pack (pure data
    movement with runtime offsets) are all host-side.
All DRAM operands are host-pre-arranged so every DMA is contiguous per
partition (8KB+ descriptors); outputs consolidate into one [128, 11*99]
fp32 tensor written with 3 block-level DMAs.
"""

import numpy as np
import ml_dtypes

import concourse.bass as bass
import concourse.bacc as bacc
import concourse.tile as tile
import concourse.mybir as mybir
from concourse.bass_utils import run_bass_kernel_spmd

F32 = mybir.dt.float32
F16 = mybir.dt.float16
F8 = mybir.dt.float8e4
EPS = 0.0009

N_CORES = 8
NB = 256
NB_C = NB // N_CORES       # samples per core
NC = 512                   # channels
HW = 512                   # nH*nW
NT = 40                    # text steps
NCLS = 97
D = NCLS + 1

ASCALE = 256.0             # host scale on normalized A; 1/ASCALE folded into W
BLOCKS = [12, 12, 8]       # samples per block (sum = NB_C)
GROUP = 4                  # samples per feature DMA


def _mgroups(ns):
    """Row-groups of t-columns (<=120 each) for the per-group stages."""
    w = ns * NT
    out, o = [], 0
    while o < w:
        m = min(120, w - o)
        out.append((o, m))
        o += m
    return out


GTOT = sum(len(_mgroups(ns)) for ns in BLOCKS)   # 11 output groups
OCOLS = GTOT * (D + 1)                           # 99 cols per group


def build_kernel(reps=1, group=GROUP, timing_mode=False, hw_loop=0,
                 dt_in="f8", rings="sa", out_rings="sa", ft_bufs=5,
                 out_split=True, first_split=True):
    """Build + compile the per-core Bass program. Returns nc.

    timing_mode=True replaces the bulk inputs with Internal DRAM scratch so
    repeated-execution benchmarks don't pay host->device re-transfers; the
    on-device HBM traffic is identical.
    """
    nc = bacc.Bacc("TRN2", target_bir_lowering=False, debug=False,
                   enable_asserts=True, num_devices=N_CORES)
    dt_ft = {"f8": F8, "f16": F16}[dt_in]

    kind_b = "Internal" if timing_mode else "ExternalInput"
    ft = nc.dram_tensor("ft", [128, NB_C * 4 * NC], dt_ft, kind=kind_b).ap()
    at = nc.dram_tensor("at", [128, NB_C * 4 * NT], dt_ft, kind=kind_b).ap()
    wt = nc.dram_tensor("wt", [128, 4 * NC], F16, kind=kind_b).ap()
    p2 = nc.dram_tensor("p2", [128, 4 * D], F16, kind=kind_b).ap()
    oraw = nc.dram_tensor("oraw", [128, OCOLS], F32, kind="ExternalOutput").ap()

    with tile.TileContext(nc) as tc:
        with (
            tc.tile_pool(name="consts", bufs=1) as consts,
            tc.tile_pool(name="ftp", bufs=ft_bufs) as ftp,
            tc.tile_pool(name="ctp", bufs=8) as ctp,
            tc.tile_pool(name="sqp", bufs=2) as sqp,
            tc.tile_pool(name="outp", bufs=3) as outp,
            tc.tile_pool(name="ps_ct", bufs=4, space="PSUM") as ps_ct,
            tc.tile_pool(name="ps_h", bufs=2, space="PSUM") as ps_h,
            tc.tile_pool(name="ps_p", bufs=2, space="PSUM") as ps_p,
        ):
            emap = {"s": nc.sync, "a": nc.scalar, "g": nc.gpsimd}
            ring_eng = [emap[ch] for ch in rings]
            oring_eng = [emap[ch] for ch in out_rings]

            def emit():
                _emit_once(nc, tc, consts, ftp, ctp, sqp, outp,
                           ps_ct, ps_h, ps_p,
                           ft, at, wt, p2, oraw, dt_ft,
                           group, ring_eng, oring_eng, out_split, first_split)

            if hw_loop:
                with tc.For_i(0, hw_loop, 1):
                    emit()
            else:
                for _ in range(reps):
                    emit()
    nc.compile()
    return nc


def _emit_once(nc, tc, consts, ftp, ctp, sqp, outp,
               ps_ct, ps_h, ps_p,
               ft, at, wt, p2, oraw, dt_ft, group, ring_eng, oring_eng,
               out_split=True, first_split=True):
    nring = len(ring_eng)

    # ---- input loads, startup-ordered -----------------------------------
    # Issue order per queue matters: the first feature tile and the first
    # half of the attention maps gate the first M1, so they go first on
    # separate queues; wt/p2 are not needed until M2 (~halfway) and load
    # behind the early feature tiles.
    n_ft = sum(ns // group for ns in BLOCKS)
    ft_tiles = [ftp.tile([128, group * 4, NC], dt_ft, tag="ft",
                         name=f"ftile{j}") for j in range(n_ft)]
    ft_offs = []
    for bi, ns in enumerate(BLOCKS):
        s_base = sum(BLOCKS[:bi])
        for j in range(ns // group):
            ft_offs.append((s_base + j * group) * 4 * NC)

    at_all = consts.tile([128, NB_C * 4, NT], dt_ft, tag="at_all")
    wt_sb = consts.tile([128, 4 * NC], F16, tag="wt")
    p2_sb = consts.tile([128, 4 * D], F16, tag="p2")
    half = NB_C * 2
    gsz = group * 4 * NC

    def load_ft(j, eng):
        if j == 0 and first_split:
            h = gsz // 2
            eng.dma_start(out=ft_tiles[0][:, :group * 2, :],
                          in_=ft[:, :h])
            eng.dma_start(out=ft_tiles[0][:, group * 2:, :],
                          in_=ft[:, h:gsz])
        else:
            off = ft_offs[j]
            eng.dma_start(out=ft_tiles[j][:], in_=ft[:, off:off + gsz])

    issue = [
        (0, lambda e: load_ft(0, e)),
        (1 % nring, lambda e: e.dma_start(out=at_all[:, :half, :],
                                          in_=at[:, :half * NT])),
        (1 % nring, lambda e: load_ft(1, e)),
        (2 % nring, lambda e: load_ft(2, e)),
        (0, lambda e: e.dma_start(out=wt_sb[:], in_=wt[:])),
        (1 % nring, lambda e: e.dma_start(out=p2_sb[:], in_=p2[:])),
        (2 % nring, lambda e: e.dma_start(out=at_all[:, half:, :],
                                          in_=at[:, half * NT:])),
    ]
    for j in range(3, n_ft):
        issue.append((j % nring, lambda e, j=j: load_ft(j, e)))
    for r, thunk in issue:
        thunk(ring_eng[r])

    # ---- main loop over sample blocks -----------------------------------
    s0 = 0
    gi = 0          # global output-group index
    fj = 0          # feature-tile index
    for ns in BLOCKS:
        w = ns * NT
        ftg = [ft_tiles[fj + j] for j in range(ns // group)]
        fj += ns // group

        # M1: Craw^T accumulated into 4 psum banks, one 40-col slice/sample
        ct_ps = [ps_ct.tile([128, 512], F32, tag="ct", name=f"ct_ps{jj}")
                 for jj in range(4)]
        for sl in range(ns):
            ftile = ftg[sl // group]
            h = sl % group
            abase = (s0 + sl) * 4
            for jj in range(4):
                for kk in range(4):
                    nc.tensor.matmul(
                        ct_ps[jj][:, sl * NT:(sl + 1) * NT],
                        ftile[:, h * 4 + kk, jj * 128:(jj + 1) * 128],
                        at_all[:, abase + kk, :],
                        start=(kk == 0), stop=(kk == 3))

        # cast to fp16 for the downstream matmuls
        ct_sb = []
        for jj in range(4):
            t = ctp.tile([128, 480], F16, tag="ct_sb")
            nc.vector.tensor_copy(t[:, :w], ct_ps[jj][:, :w])
            ct_sb.append(t)

        odense = outp.tile([128, 4 * (D + 1)], F32, tag="od")
        ng = 0
        for (o, m) in _mgroups(ns):
            col = ng * (D + 1)
            # M2: h0[t, c'] for this row-group; only its row-norms survive
            h2_ps = ps_h.tile([128, 512], F32, tag="h2")
            for kk in range(4):
                nc.tensor.matmul(h2_ps[:m, :], ct_sb[kk][:, o:o + m],
                                 wt_sb[:, kk * NC:(kk + 1) * NC],
                                 start=(kk == 0), stop=(kk == 3))
            sq = sqp.tile([128, 512], F32, tag="sq")
            nc.scalar.activation(sq[:m, :], h2_ps[:m, :],
                                 mybir.ActivationFunctionType.Square,
                                 accum_out=odense[:m, col + D:col + D + 1])

            # M3: cfP-without-bias (97 cols) + h0.b (col 97)
            p_ps = ps_p.tile([128, D], F32, tag="p")
            for kk in range(4):
                nc.tensor.matmul(p_ps[:m, :], ct_sb[kk][:, o:o + m],
                                 p2_sb[:, kk * D:(kk + 1) * D],
                                 start=(kk == 0), stop=(kk == 3))
            nc.vector.tensor_copy(odense[:m, col:col + D], p_ps[:m, :])
            ng += 1

        if out_split:
            done = 0
            while done < ng:
                take = min(2, ng - done)
                c0 = done * (D + 1)
                oring_eng[(gi + done) % len(oring_eng)].dma_start(
                    out=oraw[:, (gi + done) * (D + 1):
                             (gi + done + take) * (D + 1)],
                    in_=odense[:, c0:c0 + take * (D + 1)])
                done += take
        else:
            oring_eng[gi % len(oring_eng)].dma_start(
                out=oraw[:, gi * (D + 1):(gi + ng) * (D + 1)],
                in_=odense[:, :ng * (D + 1)])
        gi += ng
        s0 += ns


# ---- host side -----------------------------------------------------------

def host_prep(feature, A, protos, W, b, np_dt=ml_dtypes.float8_e4m3):
    """Build the 8 per-core input maps (host-side layout + algebra prep)."""
    f3 = feature.reshape(NB, NC, HW).transpose(0, 2, 1)      # [NB, HW, NC]
    a3 = A.reshape(NB, NT, HW)
    s = a3.sum(axis=2, dtype=np.float64)                     # [NB, NT]
    a_n = (ASCALE * a3 / s[:, :, None]).astype(np.float32)
    a_n = a_n.transpose(0, 2, 1)                             # [NB, HW, NT]

    wt2 = (W.T.astype(np.float64) / ASCALE)                  # [c, c']
    wt_dev = wt2.reshape(4, 128, NC).transpose(1, 0, 2).reshape(128, 4 * NC)
    p2t = wt2 @ protos.T.astype(np.float64)                  # [c, 97]
    hbcol = wt2 @ b.astype(np.float64)                       # [c]
    p2full = np.concatenate([p2t, hbcol[:, None]], axis=1)   # [c, 98]
    p2_dev = p2full.reshape(4, 128, D).transpose(1, 0, 2).reshape(128, 4 * D)

    in_maps = []
    for c in range(N_CORES):
        sl = slice(c * NB_C, (c + 1) * NB_C)
        fc = f3[sl].reshape(NB_C, 4, 128, NC).transpose(2, 0, 1, 3)
        ac = a_n[sl].reshape(NB_C, 4, 128, NT).transpose(2, 0, 1, 3)
        in_maps.append(dict(
            ft=np.ascontiguousarray(fc.reshape(128, NB_C * 4 * NC)).astype(np_dt),
            at=np.ascontiguousarray(ac.reshape(128, NB_C * 4 * NT)).astype(np_dt),
            wt=wt_dev.astype(np.float16),
            p2=p2_dev.astype(np.float16),
        ))
    return in_maps


def host_finish(oraws, protos, W, b, ALPHA, UNK_SCR):
    """Decode the per-core raw outputs into dense [NB*NT, D] cls/cos."""
    pb = (protos.astype(np.float64) @ b.astype(np.float64)).astype(np.float32)
    bb = float(b.astype(np.float64) @ b.astype(np.float64))
    alpha = float(np.asarray(ALPHA).reshape(-1)[0])
    unk = float(np.asarray(UNK_SCR).reshape(-1)[0])

    groups = []     # (flat_row_base, m, gi)
    s0, gi = 0, 0
    for ns in BLOCKS:
        for (o, m) in _mgroups(ns):
            groups.append((s0 * NT + o, m, gi))
            gi += 1
        s0 += ns

    dense_cls = np.empty((NB * NT, D), np.float32)
    dense_cos = np.empty((NB * NT, D), np.float32)
    for c, raw in enumerate(oraws):
        r = raw.reshape(128, GTOT, D + 1)
        base_c = c * NB_C * NT
        for row0, m, g in groups:
            cf = r[:m, g, :NCLS] + pb[None, :]
            hb = r[:m, g, NCLS]
            h2 = r[:m, g, NCLS + 1]
            hnorm = np.sqrt(np.maximum(h2 + 2.0 * hb + bb, 0.0))
            rows = slice(base_c + row0, base_c + row0 + m)
            dense_cls[rows, :NCLS] = cf * alpha
            dense_cos[rows, :NCLS] = cf / (hnorm[:, None] + EPS)
    dense_cls[:, NCLS] = unk
    dense_cos[:, NCLS] = unk
    return dense_cls, dense_cos


def host_pack(dense_cls, dense_cos, textLength):
    """Ragged per-sample packing (matches reference.pack)."""
    usedLen = np.minimum(textLength.astype(np.int64), NT)
    offsets = np.cumsum(usedLen) - usedLen
    b_idx, t_idx = np.nonzero(np.arange(NT)[None, :] < usedLen[:, None])
    out_cls = np.zeros((NB * NT, D), np.float32)
    out_cos = np.zeros((NB * NT, D), np.float32)
    dest = offsets[b_idx] + t_idx
    src = b_idx * NT + t_idx
    out_cls[dest] = dense_cls[src]
    out_cos[dest] = dense_cos[src]
    return out_cls, out_cos


_NC_CACHE = {}


def _get_nc(**kw):
    key = tuple(sorted(kw.items()))
    if key not in _NC_CACHE:
        _NC_CACHE[key] = build_kernel(**kw)
    return _NC_CACHE[key]


FINAL_CFG = dict(dt_in="f8", rings="sa", out_rings="sa")


def kernel(feature, A, protos, W, b, ALPHA, UNK_SCR, textLength):
    feature = np.asarray(feature, np.float32)
    A = np.asarray(A, np.float32)
    protos = np.asarray(protos, np.float32)
    W = np.asarray(W, np.float32)
    b = np.asarray(b, np.float32)
    in_maps = host_prep(feature, A, protos, W, b)
    nc = _get_nc(**FINAL_CFG)
    res = None
    for attempt in range(3):
        try:
            res = run_bass_kernel_spmd(nc, in_maps, core_ids=list(range(N_CORES)))
            break
        except Exception:  # noqa: BLE001 - transient device/tunnel hiccups
            if attempt == 2:
                raise
            import time as _time
            _time.sleep(30)
    oraws = [res.results[c]["oraw"] for c in range(N_CORES)]
    dense_cls, dense_cos = host_finish(oraws, protos, W, b, ALPHA, UNK_SCR)
    return host_pack(dense_cls, dense_cos, np.asarray(textLength))
